# revision 13
# baseline (speedup 1.0000x reference)
"""Chamfer distance loss kernel for Trainium2 (8 NeuronCores, SPMD).

Problem: nn_ChamferDistLoss — inputs pt_src, pt_ref, points_src, points_ref,
all [B=4, N=4096, 3] fp32.  Output: scalar

    loss = chamfer(pt_src, pt_ref)            # symmetric
         + chamfer_single(pt_src, points_src) # one-sided
         + chamfer_single(pt_ref, points_ref) # one-sided

Every term decomposes into one-sided tasks "sum_i min_j ||a_i - b_j||" over
[4096 x 4096] pairs.  There are 16 (direction, batch) tasks; each core gets 2.

Device algorithm per task (A, B both [4096, 3]):
  d2[i,j] = a2_i + b2_j - 2<a_i,b_j> via ONE K=16 fp16 matmul per [128 x 512]
  tile (exact hi/lo fp16 split, all 4 cross terms), accumulated in fp32 PSUM.

  The min-reduction is the bottleneck (PSUM reads are 1 elem/cycle/lane), so
  it is split across BOTH psum-capable engines running in parallel:
    - ScalarE (ACT, 1.2 GHz) copies granules 0,1 of each m-block from PSUM to
      fp16 SBUF (C0, C1).
    - VectorE (DVE, 0.96 GHz) drains granules 2,3 with a fused
      tensor_tensor_scan (min, min):  state = min(psum[t], state, C[t]) whose
      output is written through a stride-0 broadcast AP so the LAST write —
      the complete running min — lands directly in the collect column.  This
      simultaneously consumes the ACT copies: no separate fold/reduce pass.
  Each m-block yields 2 partial row-min columns; relu/sqrt/sum happen on the
  host (a few KB per core), which also removes the sqrt table-load.
"""

import numpy as np

import concourse.bass as bass
import concourse.bacc as bacc
import concourse.tile as tile
from concourse import mybir
from concourse import bass_utils

F32 = mybir.dt.float32
F16 = mybir.dt.float16
AX = mybir.AxisListType
OP = mybir.AluOpType
ACT = mybir.ActivationFunctionType

NPTS = 4096
P = 128
GRP = NPTS // P          # 32 points per partition in the [128, 96] load
NBLK = NPTS // P         # 32 M-blocks per task
NTASK = 2                # tasks per core
K = 16                   # matmul contraction rows
GRAN = 1024              # psum granule (2 banks)
HALF = 512               # matmul moving free dim (1 bank)
NSLOT = 3                # collect column slots per m-block (max DVE scans)
NCOL = NTASK * NBLK * NSLOT
BIG = 30000.0            # neutral element for min (all real d2 << this)


def _rows_dma(nc, R, r0, nrows, src_cols):
    """DMA rows of R: R[r0+r, 32p+c] = src_cols[p, 32r+c].  One HWDGE
    (nc.sync) DMA per row — BIR requires the partition dim first in each AP,
    so rows can't batch; HWDGE keeps Q7 descriptor-generation off the
    critical path entirely (the baseline used SWDGE: ~1.1us of gpsimd per
    DMA x 56)."""
    for r in range(nrows):
        nc.sync.dma_start(
            R[r0 + r : r0 + r + 1, :].rearrange("r (p c) -> r p c", p=P),
            src_cols[:, r * GRP : (r + 1) * GRP],
        )


def _build_side(tc, pool, wpool, t_dram, ones_ap, side, tag):
    """Load one [4096, 3] input, build its [16, 4096] fp16 matmul operand."""
    nc = tc.nc
    L = wpool.tile([P, 3 * GRP], F32, tag="ld")
    # contiguous per-partition load: partition p holds points p*32 .. p*32+31
    nc.sync.dma_start(L[:], t_dram.rearrange("(p g) k -> p (g k)", p=P))

    # coordinate source in (k, g) order; b side scaled by -2 (exact in fp16)
    if side == "a":
        base_kg = L[:].rearrange("p (g k) -> p k g", k=3)
    else:
        t2 = wpool.tile([P, 3 * GRP], F32, tag="t2")
        nc.gpsimd.tensor_scalar_mul(t2[:], L[:], -2.0)
        base_kg = t2[:].rearrange("p (g k) -> p k g", k=3)

    # hi/lo fp16 split of the (scaled) coordinates, blocks [xh yh zh xl yl zl]
    S6 = wpool.tile([P, 6 * GRP], F16, tag="s6")
    hi3 = S6[:, 0 : 3 * GRP].rearrange("p (k g) -> p k g", k=3)
    lo3 = S6[:, 3 * GRP : 6 * GRP].rearrange("p (k g) -> p k g", k=3)
    nc.gpsimd.tensor_copy(hi3, base_kg)
    nc.gpsimd.tensor_sub(lo3, base_kg, hi3)

    # squared norm n2 = x^2+y^2+z^2 (fp32), then hi/lo split
    sq = wpool.tile([P, 3 * GRP], F32, tag="sq")
    nc.gpsimd.tensor_mul(sq[:], L[:], L[:])
    sq_kg = sq[:].rearrange("p (g k) -> p k g", k=3)
    n2 = wpool.tile([P, GRP], F32, tag="n2")
    nc.gpsimd.tensor_add(n2[:], sq_kg[:, 0:1, :], sq_kg[:, 1:2, :])
    nc.gpsimd.tensor_add(n2[:], n2[:], sq_kg[:, 2:3, :])
    Sn = wpool.tile([P, 2 * GRP], F16, tag="sn")
    nc.gpsimd.tensor_copy(Sn[:, 0:GRP], n2[:])
    nc.gpsimd.tensor_sub(Sn[:, GRP : 2 * GRP], n2[:], Sn[:, 0:GRP])

    # assemble R [16, 4096] fp16; free index n = p*32 + c
    R = pool.tile([K, NPTS], F16, tag=tag)
    if side == "a":
        _rows_dma(nc, R, 0, 6, S6[:, 0 : 6 * GRP])   # rows 0-5:  h h h l l l
        _rows_dma(nc, R, 6, 6, S6[:, 0 : 6 * GRP])   # rows 6-11: h h h l l l
        nc.sync.dma_start(R[12:14, :], ones_ap)      # rows 12-13: 1 1
        _rows_dma(nc, R, 14, 2, Sn[:])               # rows 14-15: a2h a2l
    else:
        _rows_dma(nc, R, 0, 6, S6[:, 0 : 6 * GRP])   # rows 0-5:  h h h l l l
        _rows_dma(nc, R, 6, 3, S6[:, 3 * GRP : 6 * GRP])  # rows 6-8:  l l l
        _rows_dma(nc, R, 9, 3, S6[:, 0 : 3 * GRP])   # rows 9-11: h h h
        _rows_dma(nc, R, 12, 2, Sn[:])               # rows 12-13: b2h b2l
        nc.sync.dma_start(R[14:16, :], ones_ap)      # rows 14-15: 1 1
    return R


def chamfer_core_kernel(tc, out_ap, in_aps, ones_ap):
    """Per-core program: 2 tasks, each sum_i min_j ||a_i - b_j||."""
    nc = tc.nc
    from contextlib import ExitStack

    with ExitStack() as ctx:
        const_pool = ctx.enter_context(tc.tile_pool(name="const", bufs=1))
        work_pool = ctx.enter_context(tc.tile_pool(name="work", bufs=4))
        cbuf_pool = ctx.enter_context(tc.tile_pool(name="cbuf", bufs=3))

        sides = []
        for t in range(NTASK):
            Ra = _build_side(
                tc, const_pool, work_pool, in_aps[2 * t], ones_ap, "a", f"Ra{t}"
            )
            Rb = _build_side(
                tc, const_pool, work_pool, in_aps[2 * t + 1], ones_ap, "b", f"Rb{t}"
            )
            sides.append((Ra, Rb))

        # collect[p, (t*32+m)*3 + s] = partial row-min s of m-block m, task t;
        # unwritten slots stay BIG.  Host: min over slots / relu / sqrt / sum.
        collect = const_pool.tile([P, NCOL], F32)
        nc.vector.memset(collect[:], BIG)
        DUM = const_pool.tile([P, GRAN], F16)
        nc.vector.memset(DUM[:], BIG)

        with tc.tile_pool(name="psA", bufs=2, space="PSUM") as psA, tc.tile_pool(
            name="psD", bufs=2, space="PSUM"
        ) as psD:
            for t in range(NTASK):
                Ra, Rb = sides[t]
                for m in range(NBLK):
                    lhsT = Ra[:, m * P : (m + 1) * P]
                    # HW-measured: DVE scan ~887ns vs ACT copy ~1577ns per
                    # [128,1024] granule -> mix 1-ACT/3-DVE with 2-ACT/2-DVE
                    # m-blocks at 11:32 ACT-granule ratio to balance engines.
                    nA = (1, 2, 1, 2, 1, 1, 2, 1)[m % 8]
                    Cs = []
                    for g in range(nA):  # ACT granules -> fp16 SBUF
                        ga = psA.tile([P, GRAN], F32, tag="a")
                        for h in range(2):
                            n0 = g * GRAN + h * HALF
                            nc.tensor.matmul(
                                ga[:, h * HALF : (h + 1) * HALF],
                                lhsT,
                                Rb[:, n0 : n0 + HALF],
                                start=True,
                                stop=True,
                            )
                        C = cbuf_pool.tile([P, GRAN], F16, tag=f"c{g}")
                        nc.scalar.activation(C[:], ga[:], ACT.Copy)
                        Cs.append(C)
                    # remaining granules -> DVE fused scan-drain; the first
                    # nA scans also consume the ACT copies, the rest pair
                    # with a neutral dummy (no cross-engine dependency).
                    for s, g in enumerate(range(nA, 4)):
                        gd = psD.tile([P, GRAN], F32, tag="d")
                        for h in range(2):
                            n0 = g * GRAN + h * HALF
                            nc.tensor.matmul(
                                gd[:, h * HALF : (h + 1) * HALF],
                                lhsT,
                                Rb[:, n0 : n0 + HALF],
                                start=True,
                                stop=True,
                            )
                        data1 = Cs[s][:] if s < nA else DUM[:]
                        col = (t * NBLK + m) * NSLOT + s
                        nc.vector.tensor_tensor_scan(
                            collect[:, col : col + 1].broadcast_to((P, GRAN)),
                            gd[:],
                            data1,
                            1.0e30,
                            OP.min,
                            OP.min,
                        )

        nc.sync.dma_start(out_ap, collect[:])


_CACHED = {}


def _get_program(repeats=1):
    if repeats in _CACHED:
        return _CACHED[repeats]
    nc = bacc.Bacc("TRN2", target_bir_lowering=False, debug=False, num_devices=8)
    in_names = ["a0", "b0", "a1", "b1"]
    in_aps = [
        nc.dram_tensor(n, [NPTS, 3], F32, kind="ExternalInput").ap() for n in in_names
    ]
    ones_ap = nc.dram_tensor("ones2", [2, NPTS], F16, kind="ExternalInput").ap()
    out_ap = nc.dram_tensor("out", [P, NCOL], F32, kind="ExternalOutput").ap()
    with tile.TileContext(nc) as tc:
        for _ in range(repeats):
            chamfer_core_kernel(tc, out_ap, in_aps, ones_ap)
    nc.compile()
    _CACHED[repeats] = nc
    return nc


def _shard(pt_src, pt_ref, points_src, points_ref):
    """Host-side sharding: 16 (direction, batch) tasks -> 8 cores x 2 tasks."""
    ones2 = np.ones((2, NPTS), dtype=np.float16)
    in_maps = []
    for c in range(8):
        if c < 4:
            b = c
            m = {"a0": pt_src[b], "b0": pt_ref[b], "a1": pt_ref[b], "b1": pt_src[b]}
        else:
            b = c - 4
            m = {
                "a0": pt_src[b],
                "b0": points_src[b],
                "a1": pt_ref[b],
                "b1": points_ref[b],
            }
        m = {k: np.ascontiguousarray(v, dtype=np.float32) for k, v in m.items()}
        m["ones2"] = ones2
        in_maps.append(m)
    return in_maps


def _get_runner(repeats=1):
    """Cached jitted executor — the NEFF is loaded once; later calls only
    dispatch an execute (unlike run_bass_kernel_spmd, which rebuilds the
    jit closure and re-loads the NEFF on every call)."""
    key = ("runner", repeats)
    if key in _CACHED:
        return _CACHED[key]
    import jax
    from jax.sharding import Mesh, PartitionSpec
    from jax.experimental.shard_map import shard_map
    from concourse import bass2jax, mybir as _mb

    bass2jax.install_neuronx_cc_hook()
    nc = _get_program(repeats)
    n_cores = 8

    partition_name = (
        nc.partition_id_tensor.name if nc.partition_id_tensor is not None else None
    )
    in_names, out_names, out_avals, zero_shapes = [], [], [], []
    for alloc in nc.m.functions[0].allocations:
        if not isinstance(alloc, _mb.MemoryLocationSet):
            continue
        name = alloc.memorylocations[0].name
        if alloc.kind == "ExternalInput":
            if name != partition_name:
                in_names.append(name)
        elif alloc.kind == "ExternalOutput":
            out_names.append(name)
            shape = tuple(alloc.tensor_shape)
            dtype = _mb.dt.np(alloc.dtype)
            out_avals.append(jax.core.ShapedArray(shape, dtype))
            zero_shapes.append((shape, dtype))
    n_params = len(in_names)
    all_names = in_names + out_names
    if partition_name is not None:
        all_names = all_names + [partition_name]
    donate = tuple(range(n_params, n_params + len(out_names)))

    def _body(*args):
        operands = list(args)
        if partition_name is not None:
            operands.append(bass2jax.partition_id_tensor())
        outs = bass2jax._bass_exec_p.bind(
            *operands,
            out_avals=tuple(out_avals),
            in_names=tuple(all_names),
            out_names=tuple(out_names),
            lowering_input_output_aliases=(),
            sim_require_finite=True,
            sim_require_nnan=True,
            nc=nc,
        )
        return tuple(outs)

    devices = jax.devices()[:n_cores]
    mesh = Mesh(np.asarray(devices), ("core",))
    in_specs = (PartitionSpec("core"),) * (n_params + len(out_names))
    out_specs = (PartitionSpec("core"),) * len(out_names)
    sharded = jax.jit(
        shard_map(
            _body, mesh=mesh, in_specs=in_specs, out_specs=out_specs, check_rep=False
        ),
        donate_argnums=donate,
        keep_unused=True,
    )

    def run(in_maps):
        concat_in = [
            np.concatenate([in_maps[c][nm] for c in range(n_cores)], axis=0)
            for nm in in_names
        ]
        concat_zeros = [
            np.zeros((n_cores * s[0], *s[1:]), d) for (s, d) in zero_shapes
        ]
        out_arrs = sharded(*concat_in, *concat_zeros)
        return [
            {
                nm: np.asarray(out_arrs[i]).reshape(n_cores, *out_avals[i].shape)[c]
                for i, nm in enumerate(out_names)
            }
            for c in range(n_cores)
        ]

    _CACHED[key] = run
    return run


def kernel(pt_src, pt_ref, points_src, points_ref, _repeats=1):
    run = _get_runner(_repeats)
    in_maps = _shard(pt_src, pt_ref, points_src, points_ref)
    results = run(in_maps)
    total = np.float64(0.0)
    for r in results:
        arr = r["out"].astype(np.float64)  # [128, NCOL]
        d2 = arr.reshape(P, NTASK, NBLK, NSLOT).min(axis=3)  # combine slots
        total += np.sqrt(np.maximum(d2, 0.0)).sum()
    out = np.float32(total / (4 * NPTS))
    return np.asarray(out, dtype=np.float32)


# revision 14
# speedup vs baseline: 1.4380x; 1.4380x over previous
"""Chamfer distance loss kernel for Trainium2 (8 NeuronCores, SPMD).

Problem: nn_ChamferDistLoss — inputs pt_src, pt_ref, points_src, points_ref,
all [B=4, N=4096, 3] fp32.  Output: scalar

    loss = chamfer(pt_src, pt_ref)            # symmetric
         + chamfer_single(pt_src, points_src) # one-sided
         + chamfer_single(pt_ref, points_ref) # one-sided

Every term decomposes into one-sided tasks "sum_i min_j ||a_i - b_j||" over
[4096 x 4096] pairs.  There are 16 (direction, batch) tasks; each core gets 2.

Device algorithm per task (A, B both [4096, 3]):
  d2[i,j] = a2_i + b2_j - 2<a_i,b_j> via ONE K=16 fp16 matmul per [128 x 512]
  tile (exact hi/lo fp16 split, all 4 cross terms), accumulated in fp32 PSUM.

  The min-reduction is the bottleneck (PSUM reads are 1 elem/cycle/lane), so
  it is split across BOTH psum-capable engines running in parallel:
    - ScalarE (ACT, 1.2 GHz) copies granules 0,1 of each m-block from PSUM to
      fp16 SBUF (C0, C1).
    - VectorE (DVE, 0.96 GHz) drains granules 2,3 with a fused
      tensor_tensor_scan (min, min):  state = min(psum[t], state, C[t]) whose
      output is written through a stride-0 broadcast AP so the LAST write —
      the complete running min — lands directly in the collect column.  This
      simultaneously consumes the ACT copies: no separate fold/reduce pass.
  Each m-block yields 2 partial row-min columns; relu/sqrt/sum happen on the
  host (a few KB per core), which also removes the sqrt table-load.
"""

import numpy as np

import concourse.bass as bass
import concourse.bacc as bacc
import concourse.tile as tile
from concourse import mybir
from concourse import bass_utils

F32 = mybir.dt.float32
F16 = mybir.dt.float16
AX = mybir.AxisListType
OP = mybir.AluOpType
ACT = mybir.ActivationFunctionType

NPTS = 4096
P = 128
GRP = NPTS // P          # 32 points per partition in the [128, 96] load
NBLK = NPTS // P         # 32 M-blocks per task
NTASK = 2                # tasks per core
K = 16                   # matmul contraction rows
GRAN = 1024              # psum granule (2 banks)
HALF = 512               # matmul moving free dim (1 bank)
NSLOT = 3                # collect column slots per m-block (max DVE scans)
NCOL = NTASK * NBLK * NSLOT
BIG = 30000.0            # neutral element for min (all real d2 << this)


def _rows_dma(nc, R, r0, nrows, src_cols):
    """DMA rows of R: R[r0+r, 32p+c] = src_cols[p, 32r+c].  One HWDGE
    (nc.sync) DMA per row — BIR requires the partition dim first in each AP,
    so rows can't batch; HWDGE keeps Q7 descriptor-generation off the
    critical path entirely (the baseline used SWDGE: ~1.1us of gpsimd per
    DMA x 56)."""
    for r in range(nrows):
        nc.sync.dma_start(
            R[r0 + r : r0 + r + 1, :].rearrange("r (p c) -> r p c", p=P),
            src_cols[:, r * GRP : (r + 1) * GRP],
        )


def _build_side(tc, pool, wpool, t_dram, ones_ap, side, tag):
    """Load one [4096, 3] input, build its [16, 4096] fp16 matmul operand."""
    nc = tc.nc
    L = wpool.tile([P, 3 * GRP], F32, tag="ld")
    # contiguous per-partition load: partition p holds points p*32 .. p*32+31
    nc.sync.dma_start(L[:], t_dram.rearrange("(p g) k -> p (g k)", p=P))

    # coordinate source in (k, g) order; b side scaled by -2 (exact in fp16)
    if side == "a":
        base_kg = L[:].rearrange("p (g k) -> p k g", k=3)
    else:
        t2 = wpool.tile([P, 3 * GRP], F32, tag="t2")
        nc.gpsimd.tensor_scalar_mul(t2[:], L[:], -2.0)
        base_kg = t2[:].rearrange("p (g k) -> p k g", k=3)

    # hi/lo fp16 split of the (scaled) coordinates, blocks [xh yh zh xl yl zl]
    S6 = wpool.tile([P, 6 * GRP], F16, tag="s6")
    hi3 = S6[:, 0 : 3 * GRP].rearrange("p (k g) -> p k g", k=3)
    lo3 = S6[:, 3 * GRP : 6 * GRP].rearrange("p (k g) -> p k g", k=3)
    nc.gpsimd.tensor_copy(hi3, base_kg)
    nc.gpsimd.tensor_sub(lo3, base_kg, hi3)

    # squared norm n2 = x^2+y^2+z^2 (fp32), then hi/lo split
    sq = wpool.tile([P, 3 * GRP], F32, tag="sq")
    nc.gpsimd.tensor_mul(sq[:], L[:], L[:])
    sq_kg = sq[:].rearrange("p (g k) -> p k g", k=3)
    n2 = wpool.tile([P, GRP], F32, tag="n2")
    nc.gpsimd.tensor_add(n2[:], sq_kg[:, 0:1, :], sq_kg[:, 1:2, :])
    nc.gpsimd.tensor_add(n2[:], n2[:], sq_kg[:, 2:3, :])
    Sn = wpool.tile([P, 2 * GRP], F16, tag="sn")
    nc.gpsimd.tensor_copy(Sn[:, 0:GRP], n2[:])
    nc.gpsimd.tensor_sub(Sn[:, GRP : 2 * GRP], n2[:], Sn[:, 0:GRP])

    # assemble R [16, 4096] fp16; free index n = p*32 + c
    R = pool.tile([K, NPTS], F16, tag=tag)
    if side == "a":
        _rows_dma(nc, R, 0, 6, S6[:, 0 : 6 * GRP])   # rows 0-5:  h h h l l l
        _rows_dma(nc, R, 6, 6, S6[:, 0 : 6 * GRP])   # rows 6-11: h h h l l l
        nc.sync.dma_start(R[12:14, :], ones_ap)      # rows 12-13: 1 1
        _rows_dma(nc, R, 14, 2, Sn[:])               # rows 14-15: a2h a2l
    else:
        _rows_dma(nc, R, 0, 6, S6[:, 0 : 6 * GRP])   # rows 0-5:  h h h l l l
        _rows_dma(nc, R, 6, 3, S6[:, 3 * GRP : 6 * GRP])  # rows 6-8:  l l l
        _rows_dma(nc, R, 9, 3, S6[:, 0 : 3 * GRP])   # rows 9-11: h h h
        _rows_dma(nc, R, 12, 2, Sn[:])               # rows 12-13: b2h b2l
        nc.sync.dma_start(R[14:16, :], ones_ap)      # rows 14-15: 1 1
    return R


def chamfer_core_kernel(tc, out_ap, in_aps, ones_ap):
    """Per-core program: 2 tasks, each sum_i min_j ||a_i - b_j||."""
    nc = tc.nc
    from contextlib import ExitStack

    with ExitStack() as ctx:
        const_pool = ctx.enter_context(tc.tile_pool(name="const", bufs=1))
        work_pool = ctx.enter_context(tc.tile_pool(name="work", bufs=4))
        cbuf_pool = ctx.enter_context(tc.tile_pool(name="cbuf", bufs=3))

        sides = []
        for t in range(NTASK):
            Ra = _build_side(
                tc, const_pool, work_pool, in_aps[2 * t], ones_ap, "a", f"Ra{t}"
            )
            Rb = _build_side(
                tc, const_pool, work_pool, in_aps[2 * t + 1], ones_ap, "b", f"Rb{t}"
            )
            sides.append((Ra, Rb))

        # collect[p, (t*32+m)*3 + s] = partial row-min s of m-block m, task t;
        # unwritten slots stay BIG.  Host: min over slots / relu / sqrt / sum.
        collect = const_pool.tile([P, NCOL], F32)
        nc.vector.memset(collect[:], BIG)
        DUM = const_pool.tile([P, GRAN], F16)
        nc.vector.memset(DUM[:], BIG)

        with tc.tile_pool(name="psA", bufs=2, space="PSUM") as psA, tc.tile_pool(
            name="psD", bufs=2, space="PSUM"
        ) as psD:
            for t in range(NTASK):
                Ra, Rb = sides[t]
                for m in range(NBLK):
                    lhsT = Ra[:, m * P : (m + 1) * P]
                    # HW-measured: DVE scan ~887ns vs ACT copy ~1577ns per
                    # [128,1024] granule -> alternate 1-ACT/3-DVE with
                    # 2-ACT/2-DVE m-blocks to balance the engines.
                    nA = 1 if (m % 2 == 0) else 2
                    Cs = []
                    for g in range(nA):  # ACT granules -> fp16 SBUF
                        ga = psA.tile([P, GRAN], F32, tag="a")
                        for h in range(2):
                            n0 = g * GRAN + h * HALF
                            nc.tensor.matmul(
                                ga[:, h * HALF : (h + 1) * HALF],
                                lhsT,
                                Rb[:, n0 : n0 + HALF],
                                start=True,
                                stop=True,
                            )
                        C = cbuf_pool.tile([P, GRAN], F16, tag=f"c{g}")
                        nc.scalar.activation(C[:], ga[:], ACT.Copy)
                        Cs.append(C)
                    # remaining granules -> DVE fused scan-drain; the first
                    # nA scans also consume the ACT copies, the rest pair
                    # with a neutral dummy (no cross-engine dependency).
                    for s, g in enumerate(range(nA, 4)):
                        gd = psD.tile([P, GRAN], F32, tag="d")
                        for h in range(2):
                            n0 = g * GRAN + h * HALF
                            nc.tensor.matmul(
                                gd[:, h * HALF : (h + 1) * HALF],
                                lhsT,
                                Rb[:, n0 : n0 + HALF],
                                start=True,
                                stop=True,
                            )
                        data1 = Cs[s][:] if s < nA else DUM[:]
                        col = (t * NBLK + m) * NSLOT + s
                        nc.vector.tensor_tensor_scan(
                            collect[:, col : col + 1].broadcast_to((P, GRAN)),
                            gd[:],
                            data1,
                            1.0e30,
                            OP.min,
                            OP.min,
                        )

        nc.sync.dma_start(out_ap, collect[:])


_CACHED = {}


def _get_program(repeats=1):
    if repeats in _CACHED:
        return _CACHED[repeats]
    nc = bacc.Bacc("TRN2", target_bir_lowering=False, debug=False, num_devices=8)
    in_names = ["a0", "b0", "a1", "b1"]
    in_aps = [
        nc.dram_tensor(n, [NPTS, 3], F32, kind="ExternalInput").ap() for n in in_names
    ]
    ones_ap = nc.dram_tensor("ones2", [2, NPTS], F16, kind="ExternalInput").ap()
    out_ap = nc.dram_tensor("out", [P, NCOL], F32, kind="ExternalOutput").ap()
    with tile.TileContext(nc) as tc:
        for _ in range(repeats):
            chamfer_core_kernel(tc, out_ap, in_aps, ones_ap)
    nc.compile()
    _CACHED[repeats] = nc
    return nc


def _shard(pt_src, pt_ref, points_src, points_ref):
    """Host-side sharding: 16 (direction, batch) tasks -> 8 cores x 2 tasks."""
    ones2 = np.ones((2, NPTS), dtype=np.float16)
    in_maps = []
    for c in range(8):
        if c < 4:
            b = c
            m = {"a0": pt_src[b], "b0": pt_ref[b], "a1": pt_ref[b], "b1": pt_src[b]}
        else:
            b = c - 4
            m = {
                "a0": pt_src[b],
                "b0": points_src[b],
                "a1": pt_ref[b],
                "b1": points_ref[b],
            }
        m = {k: np.ascontiguousarray(v, dtype=np.float32) for k, v in m.items()}
        m["ones2"] = ones2
        in_maps.append(m)
    return in_maps


def _get_runner(repeats=1):
    """Cached jitted executor — the NEFF is loaded once; later calls only
    dispatch an execute (unlike run_bass_kernel_spmd, which rebuilds the
    jit closure and re-loads the NEFF on every call)."""
    key = ("runner", repeats)
    if key in _CACHED:
        return _CACHED[key]
    import jax
    from jax.sharding import Mesh, PartitionSpec
    from jax.experimental.shard_map import shard_map
    from concourse import bass2jax, mybir as _mb

    bass2jax.install_neuronx_cc_hook()
    nc = _get_program(repeats)
    n_cores = 8

    partition_name = (
        nc.partition_id_tensor.name if nc.partition_id_tensor is not None else None
    )
    in_names, out_names, out_avals, zero_shapes = [], [], [], []
    for alloc in nc.m.functions[0].allocations:
        if not isinstance(alloc, _mb.MemoryLocationSet):
            continue
        name = alloc.memorylocations[0].name
        if alloc.kind == "ExternalInput":
            if name != partition_name:
                in_names.append(name)
        elif alloc.kind == "ExternalOutput":
            out_names.append(name)
            shape = tuple(alloc.tensor_shape)
            dtype = _mb.dt.np(alloc.dtype)
            out_avals.append(jax.core.ShapedArray(shape, dtype))
            zero_shapes.append((shape, dtype))
    n_params = len(in_names)
    all_names = in_names + out_names
    if partition_name is not None:
        all_names = all_names + [partition_name]
    donate = tuple(range(n_params, n_params + len(out_names)))

    def _body(*args):
        operands = list(args)
        if partition_name is not None:
            operands.append(bass2jax.partition_id_tensor())
        outs = bass2jax._bass_exec_p.bind(
            *operands,
            out_avals=tuple(out_avals),
            in_names=tuple(all_names),
            out_names=tuple(out_names),
            lowering_input_output_aliases=(),
            sim_require_finite=True,
            sim_require_nnan=True,
            nc=nc,
        )
        return tuple(outs)

    devices = jax.devices()[:n_cores]
    mesh = Mesh(np.asarray(devices), ("core",))
    in_specs = (PartitionSpec("core"),) * (n_params + len(out_names))
    out_specs = (PartitionSpec("core"),) * len(out_names)
    sharded = jax.jit(
        shard_map(
            _body, mesh=mesh, in_specs=in_specs, out_specs=out_specs, check_rep=False
        ),
        donate_argnums=donate,
        keep_unused=True,
    )

    def run(in_maps):
        concat_in = [
            np.concatenate([in_maps[c][nm] for c in range(n_cores)], axis=0)
            for nm in in_names
        ]
        concat_zeros = [
            np.zeros((n_cores * s[0], *s[1:]), d) for (s, d) in zero_shapes
        ]
        out_arrs = sharded(*concat_in, *concat_zeros)
        return [
            {
                nm: np.asarray(out_arrs[i]).reshape(n_cores, *out_avals[i].shape)[c]
                for i, nm in enumerate(out_names)
            }
            for c in range(n_cores)
        ]

    _CACHED[key] = run
    return run


def kernel(pt_src, pt_ref, points_src, points_ref, _repeats=1):
    run = _get_runner(_repeats)
    in_maps = _shard(pt_src, pt_ref, points_src, points_ref)
    results = run(in_maps)
    total = np.float64(0.0)
    for r in results:
        arr = r["out"].astype(np.float64)  # [128, NCOL]
        d2 = arr.reshape(P, NTASK, NBLK, NSLOT).min(axis=3)  # combine slots
        total += np.sqrt(np.maximum(d2, 0.0)).sum()
    out = np.float32(total / (4 * NPTS))
    return np.asarray(out, dtype=np.float32)
